# revision 7
# baseline (speedup 1.0000x reference)
"""Causal self-attention TRN2 Bass kernel.

Problem: B=4, T=2048, C=1024, H=16 heads (HD=64), torch-Linear semantics
(y = x @ W.T + b), causal + padding mask, softmax, output projection.

Sharding: 8 cores = (batch b in 0..3) x (head-half in 0..1). Each core
handles one batch and 8 heads (512 of the 1024 channels of QKV / of the
contraction dim of the output projection). The two half-cores of a batch
produce partial output projections that the host sums (plus bp).

Per-core kernel (all matmuls in float32r — full PE rate, ~1.5e-4 rel):
  Phase 1: QKV projections.
    Q^T, K^T computed head-major ([outch, T]) so attention needs no
    transposes; V computed token-major ([T, outch]) with an interleaved
    ones column per head (rowsum trick). Attention scale (1/8) and bias
    are folded in during the PSUM->SBUF copy on DVE; V bias is added via
    a K=1 ones-outer-product matmul into PSUM.
  Phase 2: flash-style causal attention per head-pair g (2 heads packed
    on PE rows 0-63 / 64-127 via tile_position for the K=64 S^T matmuls).
    S^T[k,q] = K Q^T tiles; causal masking adds -1e30 to PSUM on the
    diagonal tiles; exp on ACT (no max subtraction needed: |S|<~3);
    O_unnorm^T[d,q] plus rowsum row via [V | 1] stationary; normalization
    via reciprocal + K=1 broadcast matmul + DVE multiply into Y^T.
  Phase 3: output projection from Y^T tiles (stationary) vs Wp^T slices.
"""

import numpy as np

import concourse.mybir as mybir
import concourse.tile as tile
from concourse import bacc
from concourse.bass_utils import run_bass_kernel_spmd

F32 = mybir.dt.float32
F32R = mybir.dt.float32r
AF = mybir.ActivationFunctionType
ALU = mybir.AluOpType

B, T, C, H = 4, 2048, 1024, 16
HD = C // H          # 64
IC = C // 2          # 512 channels per core (8 heads)
NKT = T // 128       # 16 k-tiles
NQC = T // 512       # 4 q-chunks
NCT = C // 128       # 8 contraction tiles for QKV
NEG = -1.0e30
SCALE = 1.0 / np.sqrt(HD)

_CACHE = {}


def _build():
    nc = bacc.Bacc("TRN2", target_bir_lowering=False, debug=False)

    xT_d = nc.dram_tensor("xT", [C, T], F32, kind="ExternalInput").ap()
    WqT_d = nc.dram_tensor("WqT", [C, IC], F32, kind="ExternalInput").ap()
    WkT_d = nc.dram_tensor("WkT", [C, IC], F32, kind="ExternalInput").ap()
    WvT_d = nc.dram_tensor("WvT", [C, IC], F32, kind="ExternalInput").ap()
    WpT_d = nc.dram_tensor("WpT", [IC, C], F32, kind="ExternalInput").ap()
    bq_d = nc.dram_tensor("bqs", [128, 4], F32, kind="ExternalInput").ap()
    bk_d = nc.dram_tensor("bks", [128, 4], F32, kind="ExternalInput").ap()
    bv_d = nc.dram_tensor("bvr", [1, IC], F32, kind="ExternalInput").ap()
    pad_d = nc.dram_tensor("padb", [128, NKT], F32, kind="ExternalInput").ap()
    mask_d = nc.dram_tensor("maskneg", [128, 4 * 512], F32, kind="ExternalInput").ap()
    ones128_d = nc.dram_tensor("ones128", [1, 128], F32, kind="ExternalInput").ap()
    ones8_d = nc.dram_tensor("ones8", [128, 8], F32, kind="ExternalInput").ap()
    out_d = nc.dram_tensor("out", [T, C], F32, kind="ExternalOutput").ap()

    with tile.TileContext(nc) as tc:
        with tc.tile_pool(name="pp", bufs=1) as pp:
            # Persistent SBUF state
            QT = pp.tile([128, 4 * T], F32R, name="QT")     # 4 head-pair tiles
            KT = pp.tile([128, 4 * T], F32R, name="KT")
            Vt = pp.tile([128, NKT * 520], F32R, name="Vt")  # [V|1] x 8 heads
            bq_sb = pp.tile([128, 4], F32, name="bq_sb")
            bk_sb = pp.tile([128, 4], F32, name="bk_sb")
            bv_sb = pp.tile([1, IC], F32R, name="bv_sb")
            pad_sb = pp.tile([128, NKT], F32, name="pad_sb")
            ones128 = pp.tile([1, 128], F32R, name="ones128")
            nc.sync.dma_start(out=bq_sb[:], in_=bq_d)
            nc.sync.dma_start(out=bk_sb[:], in_=bk_d)
            nc.sync.dma_start(out=bv_sb[:], in_=bv_d.bitcast(F32R))
            nc.sync.dma_start(out=pad_sb[:], in_=pad_d)
            nc.sync.dma_start(out=ones128[:], in_=ones128_d.bitcast(F32R))
            # V ones columns (col 64 of each head block of width 65)
            Vr = Vt.rearrange("p (k h c) -> p k h c", k=NKT, h=8, c=65)
            for kt in range(NKT):
                nc.sync.dma_start(out=Vr[:, kt, :, 64], in_=ones8_d.bitcast(F32R))

            # ---------------- Phase 1: QKV projections ----------------
            with tc.tile_pool(name="p1", bufs=1) as p1, \
                 tc.tile_pool(name="xs", bufs=2) as xs, \
                 tc.tile_pool(name="ps1", bufs=3, space="PSUM") as ps1:
                Wq_sb = p1.tile([128, NCT * 512], F32R, name="Wq_sb")
                Wk_sb = p1.tile([128, NCT * 512], F32R, name="Wk_sb")
                Wv_sb = p1.tile([128, NCT * 512], F32R, name="Wv_sb")
                for ct in range(NCT):
                    cs = slice(ct * 128, (ct + 1) * 128)
                    fs = slice(ct * 512, (ct + 1) * 512)
                    nc.sync.dma_start(out=Wq_sb[:, fs], in_=WqT_d[cs, :].bitcast(F32R))
                    nc.sync.dma_start(out=Wk_sb[:, fs], in_=WkT_d[cs, :].bitcast(F32R))
                    nc.sync.dma_start(out=Wv_sb[:, fs], in_=WvT_d[cs, :].bitcast(F32R))

                for tch in range(4):  # T chunks of 512
                    t0 = tch * 512
                    xc = xs.tile([128, NCT * 512], F32R, name="xc", tag="xc")
                    for ct in range(NCT):
                        nc.sync.dma_start(
                            out=xc[:, ct * 512:(ct + 1) * 512],
                            in_=xT_d[ct * 128:(ct + 1) * 128, t0:t0 + 512].bitcast(F32R),
                        )
                    # Q^T and K^T: out [outch-tile(g) 128, tokens 512]
                    for g in range(4):
                        pq = ps1.tile([128, 512], F32, name="pq", tag="ps1")
                        for ct in range(NCT):
                            nc.tensor.matmul(
                                out=pq[:],
                                lhsT=Wq_sb[:, ct * 512 + g * 128: ct * 512 + (g + 1) * 128],
                                rhs=xc[:, ct * 512:(ct + 1) * 512],
                                start=(ct == 0), stop=(ct == NCT - 1),
                            )
                        nc.vector.tensor_scalar(
                            out=QT[:, g * T + t0: g * T + t0 + 512], in0=pq[:],
                            scalar1=SCALE, scalar2=bq_sb[:, g:g + 1],
                            op0=ALU.mult, op1=ALU.add,
                        )
                        pk = ps1.tile([128, 512], F32, name="pk", tag="ps1")
                        for ct in range(NCT):
                            nc.tensor.matmul(
                                out=pk[:],
                                lhsT=Wk_sb[:, ct * 512 + g * 128: ct * 512 + (g + 1) * 128],
                                rhs=xc[:, ct * 512:(ct + 1) * 512],
                                start=(ct == 0), stop=(ct == NCT - 1),
                            )
                        nc.vector.tensor_scalar(
                            out=KT[:, g * T + t0: g * T + t0 + 512], in0=pk[:],
                            scalar1=bk_sb[:, g:g + 1], scalar2=None, op0=ALU.add,
                        )
                    # V: out [token-tile 128, outch 512] (+ bias via K=1 matmul)
                    for ts in range(4):
                        kt = tch * 4 + ts
                        pv = ps1.tile([128, 512], F32, name="pv", tag="ps1")
                        for ct in range(NCT):
                            nc.tensor.matmul(
                                out=pv[:],
                                lhsT=xc[:, ct * 512 + ts * 128: ct * 512 + ts * 128 + 128],
                                rhs=Wv_sb[:, ct * 512:(ct + 1) * 512],
                                start=(ct == 0), stop=False,
                            )
                        nc.tensor.matmul(
                            out=pv[:], lhsT=ones128[:], rhs=bv_sb[:],
                            start=False, stop=True,
                        )
                        nc.vector.tensor_copy(Vr[:, kt, :, 0:64], pv[:])

            # ---------------- Phase 2: causal attention ----------------
            with tc.tile_pool(name="p23", bufs=1) as p23:
                YT = p23.tile([128, 4 * T], F32R, name="YT")
                mask_sb = p23.tile([128, 4 * 512], F32, name="mask_sb")
                Wp_sb = p23.tile([128, 4 * C], F32R, name="Wp_sb")
                nc.sync.dma_start(out=mask_sb[:], in_=mask_d)
                for g in range(4):
                    nc.sync.dma_start(
                        out=Wp_sb[:, g * C:(g + 1) * C],
                        in_=WpT_d[g * 128:(g + 1) * 128, :].bitcast(F32R),
                    )

                with tc.tile_pool(name="es", bufs=4) as es, \
                     tc.tile_pool(name="rp", bufs=2) as rp, \
                     tc.tile_pool(name="pss", bufs=2, space="PSUM") as pss, \
                     tc.tile_pool(name="pso", bufs=1, space="PSUM") as pso:
                    for g in range(4):
                        gq = g * T
                        for qc in range(NQC):
                            q0 = qc * 512
                            kmax = 4 * qc + 4
                            oA = pso.tile([65, 512], F32, name="oA", tag="oA")
                            oB = pso.tile([65, 512], F32, name="oB", tag="oB")
                            ea_l = [None] * kmax
                            eb_l = [None] * kmax
                            for kt in range(kmax + 1):
                                if kt < kmax:
                                    k0 = kt * 128
                                    sA = pss.tile([128, 512], F32, name="sA", tag="sA")
                                    sB = pss.tile([128, 512], F32, name="sB", tag="sB")
                                    nc.tensor.matmul(
                                        out=sA[:], lhsT=KT[0:64, gq + k0: gq + k0 + 128],
                                        rhs=QT[0:64, gq + q0: gq + q0 + 512],
                                        start=True, stop=True,
                                    )
                                    nc.tensor.matmul(
                                        out=sB[:], lhsT=KT[64:128, gq + k0: gq + k0 + 128],
                                        rhs=QT[64:128, gq + q0: gq + q0 + 512],
                                        start=True, stop=True, tile_position=(64, 0),
                                    )
                                    if kt >= 4 * qc:  # diagonal tile: additive causal mask
                                        t = kt - 4 * qc
                                        ms = mask_sb[:, t * 512:(t + 1) * 512]
                                        nc.vector.tensor_add(sA[:], sA[:], ms)
                                        nc.vector.tensor_add(sB[:], sB[:], ms)
                                    eA = es.tile([128, 512], F32R, name="eA", tag="eA")
                                    eB = es.tile([128, 512], F32R, name="eB", tag="eB")
                                    nc.scalar.activation(
                                        eA[:], sA[:], AF.Exp, bias=pad_sb[:, kt:kt + 1])
                                    nc.scalar.activation(
                                        eB[:], sB[:], AF.Exp, bias=pad_sb[:, kt:kt + 1])
                                    ea_l[kt], eb_l[kt] = eA, eB
                                if kt > 0:  # PV one step behind (keeps PE fed)
                                    pk_ = kt - 1
                                    vbase = pk_ * 520
                                    nc.tensor.matmul(
                                        out=oA[:], lhsT=Vt[:, vbase + 130 * g: vbase + 130 * g + 65],
                                        rhs=ea_l[pk_][:],
                                        start=(pk_ == 0), stop=(pk_ == kmax - 1),
                                    )
                                    nc.tensor.matmul(
                                        out=oB[:], lhsT=Vt[:, vbase + 130 * g + 65: vbase + 130 * g + 130],
                                        rhs=eb_l[pk_][:],
                                        start=(pk_ == 0), stop=(pk_ == kmax - 1),
                                    )
                            # epilogue: normalize by rowsum (row 64), write Y^T
                            rA = rp.tile([1, 512], F32, name="rA", tag="rA")
                            rB = rp.tile([1, 512], F32, name="rB", tag="rB")
                            nc.vector.reciprocal(rA[:], oA[64:65, :])
                            nc.vector.reciprocal(rB[:], oB[64:65, :])
                            rbA = rp.tile([64, 512], F32, name="rbA", tag="rbA")
                            rbB = rp.tile([64, 512], F32, name="rbB", tag="rbB")
                            nc.gpsimd.partition_broadcast(rbA[:], rA[:])
                            nc.gpsimd.partition_broadcast(rbB[:], rB[:])
                            nc.vector.tensor_mul(
                                YT[0:64, gq + q0: gq + q0 + 512], oA[0:64, :], rbA[:])
                            nc.vector.tensor_mul(
                                YT[64:128, gq + q0: gq + q0 + 512], oB[0:64, :], rbB[:])

                # ---------------- Phase 3: output projection ----------------
                with tc.tile_pool(name="ob", bufs=3) as obp, \
                     tc.tile_pool(name="ps3", bufs=4, space="PSUM") as ps3:
                    for tt in range(16):
                        for oc in range(2):
                            po = ps3.tile([128, 512], F32, name="po", tag="po")
                            for g in range(4):
                                nc.tensor.matmul(
                                    out=po[:],
                                    lhsT=YT[:, g * T + tt * 128: g * T + tt * 128 + 128],
                                    rhs=Wp_sb[:, g * C + oc * 512: g * C + oc * 512 + 512],
                                    start=(g == 0), stop=(g == 3),
                                )
                            ob = obp.tile([128, 512], F32, name="ob", tag="ob")
                            nc.scalar.copy(ob[:], po[:])
                            nc.sync.dma_start(
                                out=out_d[tt * 128:(tt + 1) * 128,
                                          oc * 512:(oc + 1) * 512],
                                in_=ob[:],
                            )

    nc.compile()
    return nc


def _in_maps(x, Wk, bk, Wq, bq, Wv, bv, Wp, bp, padding_mask):
    maps = []
    mask_cols = np.arange(512)[None, :]
    mask_rows = np.arange(128)[:, None]
    maskneg = np.concatenate(
        [np.where(mask_rows + 128 * t <= mask_cols, 0.0, NEG) for t in range(4)],
        axis=1,
    ).astype(np.float32)
    for core in range(8):
        b, half = divmod(core, 2)
        hs = slice(half * IC, (half + 1) * IC)
        maps.append({
            "xT": np.ascontiguousarray(x[b].T),
            "WqT": np.ascontiguousarray(Wq[hs, :].T),
            "WkT": np.ascontiguousarray(Wk[hs, :].T),
            "WvT": np.ascontiguousarray(Wv[hs, :].T),
            "WpT": np.ascontiguousarray(Wp[:, hs].T),
            "bqs": np.ascontiguousarray((bq[hs] * SCALE).reshape(4, 128).T),
            "bks": np.ascontiguousarray(bk[hs].reshape(4, 128).T),
            "bvr": bv[hs].reshape(1, IC).copy(),
            "padb": np.ascontiguousarray(
                np.where(padding_mask[b] != 0, 0.0, NEG)
                .astype(np.float32).reshape(NKT, 128).T),
            "maskneg": maskneg,
            "ones128": np.ones((1, 128), np.float32),
            "ones8": np.ones((128, 8), np.float32),
        })
    return maps


def _run(inputs, trace=False, **kw):
    if "nc" not in _CACHE:
        _CACHE["nc"] = _build()
    nc = _CACHE["nc"]
    ins = {k: np.asarray(v, dtype=np.float32) if k != "padding_mask"
           else np.asarray(v) for k, v in inputs.items()}
    maps = _in_maps(**ins)
    res = run_bass_kernel_spmd(nc, maps, core_ids=list(range(8)), trace=trace, **kw)
    bp = np.asarray(inputs["bp"], np.float32)
    y = np.empty((B, T, C), np.float32)
    for b in range(B):
        y[b] = res.results[2 * b]["out"] + res.results[2 * b + 1]["out"] + bp
    return y, res


def kernel(**inputs):
    y, _ = _run(inputs, trace=False)
    return y


# revision 8
# speedup vs baseline: 1.0949x; 1.0949x over previous
"""Causal self-attention TRN2 Bass kernel.

Problem: B=4, T=2048, C=1024, H=16 heads (HD=64), torch-Linear semantics
(y = x @ W.T + b), causal + padding mask, softmax, output projection.

Sharding: 8 cores = (batch b in 0..3) x (head-half in 0..1). Each core
handles one batch and 8 heads (512 of the 1024 channels of QKV / of the
contraction dim of the output projection). The two half-cores of a batch
produce partial output projections that the host sums (plus bp).

Per-core kernel (all matmuls in float32r — full PE rate, ~1.5e-4 rel):
  Phase 1: QKV projections.
    Q^T, K^T computed head-major ([outch, T]) so attention needs no
    transposes; V computed token-major ([T, outch]) with an interleaved
    ones column per head (rowsum trick). Attention scale (1/8) and bias
    are folded in during the PSUM->SBUF copy on DVE; V bias is added via
    a K=1 ones-outer-product matmul into PSUM.
  Phase 2: flash-style causal attention per head-pair g (2 heads packed
    on PE rows 0-63 / 64-127 via tile_position for the K=64 S^T matmuls).
    S^T[k,q] = K Q^T tiles; causal masking adds -1e30 to PSUM on the
    diagonal tiles; exp on ACT (no max subtraction needed: |S|<~3);
    O_unnorm^T[d,q] plus rowsum row via [V | 1] stationary; normalization
    via reciprocal + K=1 broadcast matmul + DVE multiply into Y^T.
  Phase 3: output projection from Y^T tiles (stationary) vs Wp^T slices.
"""

import numpy as np

import concourse.mybir as mybir
import concourse.tile as tile
from concourse import bacc
from concourse.bass_utils import run_bass_kernel_spmd

F32 = mybir.dt.float32
F32R = mybir.dt.float32r
AF = mybir.ActivationFunctionType
ALU = mybir.AluOpType

B, T, C, H = 4, 2048, 1024, 16
HD = C // H          # 64
IC = C // 2          # 512 channels per core (8 heads)
NKT = T // 128       # 16 k-tiles
NQC = T // 512       # 4 q-chunks
NCT = C // 128       # 8 contraction tiles for QKV
NEG = -1.0e30
SCALE = 1.0 / np.sqrt(HD)

_CACHE = {}


def _build():
    nc = bacc.Bacc("TRN2", target_bir_lowering=False, debug=False)

    xT_d = nc.dram_tensor("xT", [C, T], F32, kind="ExternalInput").ap()
    WqT_d = nc.dram_tensor("WqT", [C, IC], F32, kind="ExternalInput").ap()
    WkT_d = nc.dram_tensor("WkT", [C, IC], F32, kind="ExternalInput").ap()
    WvT_d = nc.dram_tensor("WvT", [C, IC], F32, kind="ExternalInput").ap()
    WpT_d = nc.dram_tensor("WpT", [IC, C], F32, kind="ExternalInput").ap()
    bq_d = nc.dram_tensor("bqs", [128, 4], F32, kind="ExternalInput").ap()
    bk_d = nc.dram_tensor("bks", [128, 4], F32, kind="ExternalInput").ap()
    bv_d = nc.dram_tensor("bvr", [1, IC], F32, kind="ExternalInput").ap()
    pad_d = nc.dram_tensor("padb", [128, NKT], F32, kind="ExternalInput").ap()
    mask_d = nc.dram_tensor("maskneg", [128, 896], F32, kind="ExternalInput").ap()
    ones128_d = nc.dram_tensor("ones128", [1, 128], F32, kind="ExternalInput").ap()
    ones8_d = nc.dram_tensor("ones8", [128, 8], F32, kind="ExternalInput").ap()
    out_d = nc.dram_tensor("out", [T, C], F32, kind="ExternalOutput").ap()

    with tile.TileContext(nc) as tc:
        with tc.tile_pool(name="pp", bufs=1) as pp:
            # Persistent SBUF state
            QT = pp.tile([128, 4 * T], F32R, name="QT")     # 4 head-pair tiles
            KT = pp.tile([128, 4 * T], F32R, name="KT")
            Vt = pp.tile([128, NKT * 520], F32R, name="Vt")  # [V|1] x 8 heads
            bq_sb = pp.tile([128, 4], F32, name="bq_sb")
            bk_sb = pp.tile([128, 4], F32, name="bk_sb")
            bv_sb = pp.tile([1, IC], F32R, name="bv_sb")
            pad_sb = pp.tile([128, NKT], F32, name="pad_sb")
            ones128 = pp.tile([1, 128], F32R, name="ones128")
            nc.sync.dma_start(out=bq_sb[:], in_=bq_d)
            nc.sync.dma_start(out=bk_sb[:], in_=bk_d)
            nc.sync.dma_start(out=bv_sb[:], in_=bv_d.bitcast(F32R))
            nc.sync.dma_start(out=pad_sb[:], in_=pad_d)
            nc.sync.dma_start(out=ones128[:], in_=ones128_d.bitcast(F32R))
            # V ones columns (col 64 of each head block of width 65)
            Vr = Vt.rearrange("p (k h c) -> p k h c", k=NKT, h=8, c=65)
            for kt in range(NKT):
                nc.sync.dma_start(out=Vr[:, kt, :, 64], in_=ones8_d.bitcast(F32R))

            # ---------------- Phase 1: QKV projections ----------------
            with tc.tile_pool(name="p1", bufs=1) as p1, \
                 tc.tile_pool(name="xs", bufs=2) as xs, \
                 tc.tile_pool(name="ps1", bufs=3, space="PSUM") as ps1:
                Wq_sb = p1.tile([128, NCT * 512], F32R, name="Wq_sb")
                Wk_sb = p1.tile([128, NCT * 512], F32R, name="Wk_sb")
                Wv_sb = p1.tile([128, NCT * 512], F32R, name="Wv_sb")
                for ct in range(NCT):
                    cs = slice(ct * 128, (ct + 1) * 128)
                    fs = slice(ct * 512, (ct + 1) * 512)
                    nc.sync.dma_start(out=Wq_sb[:, fs], in_=WqT_d[cs, :].bitcast(F32R))
                    nc.sync.dma_start(out=Wk_sb[:, fs], in_=WkT_d[cs, :].bitcast(F32R))
                    nc.sync.dma_start(out=Wv_sb[:, fs], in_=WvT_d[cs, :].bitcast(F32R))

                for tch in range(4):  # T chunks of 512
                    t0 = tch * 512
                    xc = xs.tile([128, NCT * 512], F32R, name="xc", tag="xc")
                    for ct in range(NCT):
                        nc.sync.dma_start(
                            out=xc[:, ct * 512:(ct + 1) * 512],
                            in_=xT_d[ct * 128:(ct + 1) * 128, t0:t0 + 512].bitcast(F32R),
                        )
                    # Q^T and K^T: out [outch-tile(g) 128, tokens 512]
                    for g in range(4):
                        pq = ps1.tile([128, 512], F32, name="pq", tag="ps1")
                        for ct in range(NCT):
                            nc.tensor.matmul(
                                out=pq[:],
                                lhsT=Wq_sb[:, ct * 512 + g * 128: ct * 512 + (g + 1) * 128],
                                rhs=xc[:, ct * 512:(ct + 1) * 512],
                                start=(ct == 0), stop=(ct == NCT - 1),
                            )
                        nc.vector.tensor_scalar(
                            out=QT[:, g * T + t0: g * T + t0 + 512], in0=pq[:],
                            scalar1=SCALE, scalar2=bq_sb[:, g:g + 1],
                            op0=ALU.mult, op1=ALU.add,
                        )
                        pk = ps1.tile([128, 512], F32, name="pk", tag="ps1")
                        for ct in range(NCT):
                            nc.tensor.matmul(
                                out=pk[:],
                                lhsT=Wk_sb[:, ct * 512 + g * 128: ct * 512 + (g + 1) * 128],
                                rhs=xc[:, ct * 512:(ct + 1) * 512],
                                start=(ct == 0), stop=(ct == NCT - 1),
                            )
                        nc.vector.tensor_scalar(
                            out=KT[:, g * T + t0: g * T + t0 + 512], in0=pk[:],
                            scalar1=bk_sb[:, g:g + 1], scalar2=None, op0=ALU.add,
                        )
                    # V: out [token-tile 128, outch 512] (+ bias via K=1 matmul)
                    for ts in range(4):
                        kt = tch * 4 + ts
                        pv = ps1.tile([128, 512], F32, name="pv", tag="ps1")
                        for ct in range(NCT):
                            nc.tensor.matmul(
                                out=pv[:],
                                lhsT=xc[:, ct * 512 + ts * 128: ct * 512 + ts * 128 + 128],
                                rhs=Wv_sb[:, ct * 512:(ct + 1) * 512],
                                start=(ct == 0), stop=False,
                            )
                        nc.tensor.matmul(
                            out=pv[:], lhsT=ones128[:], rhs=bv_sb[:],
                            start=False, stop=True,
                        )
                        nc.vector.tensor_copy(Vr[:, kt, :, 0:64], pv[:])

            # ---------------- Phase 2: causal attention + projection ----------
            # qc-outer / g-inner; S and PV interleaved with skew D so PE
            # stays dense while ACT exps trail; diagonal tiles trimmed to
            # their unmasked column range; projection for each q-chunk is
            # emitted right after its attention so it runs in ACT's shadow.
            D = 4
            with tc.tile_pool(name="p23", bufs=1) as p23:
                YT = p23.tile([128, 4 * T], F32R, name="YT")
                mask_sb = p23.tile([128, 896], F32, name="mask_sb")
                Wp_sb = p23.tile([128, 4 * C], F32R, name="Wp_sb")
                nc.sync.dma_start(out=mask_sb[:], in_=mask_d)
                for g in range(4):
                    nc.sync.dma_start(
                        out=Wp_sb[:, g * C:(g + 1) * C],
                        in_=WpT_d[g * 128:(g + 1) * 128, :].bitcast(F32R),
                    )

                with tc.tile_pool(name="es", bufs=D + 1) as es, \
                     tc.tile_pool(name="rp", bufs=2) as rp, \
                     tc.tile_pool(name="pss", bufs=2, space="PSUM") as pss, \
                     tc.tile_pool(name="pso", bufs=1, space="PSUM") as pso, \
                     tc.tile_pool(name="ob", bufs=3) as obp, \
                     tc.tile_pool(name="ps3", bufs=2, space="PSUM") as ps3:
                    for qc in range(NQC):
                        q0 = qc * 512
                        kmax = 4 * qc + 4
                        for g in range(4):
                            gq = g * T
                            oA = pso.tile([65, 512], F32, name="oA", tag="oA")
                            oB = pso.tile([65, 512], F32, name="oB", tag="oB")
                            ea_l = [None] * kmax
                            eb_l = [None] * kmax
                            wid_l = [None] * kmax
                            for step in range(kmax + D):
                                if step < kmax:
                                    kt = step
                                    k0 = kt * 128
                                    toff = 128 * (kt - 4 * qc) if kt >= 4 * qc else 0
                                    w = 512 - toff
                                    wid_l[kt] = toff
                                    sA = pss.tile([128, 512], F32, name="sA", tag="sA")
                                    sB = pss.tile([128, 512], F32, name="sB", tag="sB")
                                    nc.tensor.matmul(
                                        out=sA[:, toff:512],
                                        lhsT=KT[0:64, gq + k0: gq + k0 + 128],
                                        rhs=QT[0:64, gq + q0 + toff: gq + q0 + 512],
                                        start=True, stop=True,
                                    )
                                    nc.tensor.matmul(
                                        out=sB[:, toff:512],
                                        lhsT=KT[64:128, gq + k0: gq + k0 + 128],
                                        rhs=QT[64:128, gq + q0 + toff: gq + q0 + 512],
                                        start=True, stop=True, tile_position=(64, 0),
                                    )
                                    if kt >= 4 * qc:  # diagonal: additive causal mask
                                        ms = mask_sb[:, 384:384 + w]
                                        nc.vector.tensor_add(
                                            sA[:, toff:512], sA[:, toff:512], ms)
                                        nc.vector.tensor_add(
                                            sB[:, toff:512], sB[:, toff:512], ms)
                                    eA = es.tile([128, 512], F32R, name="eA", tag="eA")
                                    eB = es.tile([128, 512], F32R, name="eB", tag="eB")
                                    nc.scalar.activation(
                                        eA[:, toff:512], sA[:, toff:512], AF.Exp,
                                        bias=pad_sb[:, kt:kt + 1])
                                    nc.scalar.activation(
                                        eB[:, toff:512], sB[:, toff:512], AF.Exp,
                                        bias=pad_sb[:, kt:kt + 1])
                                    ea_l[kt], eb_l[kt] = eA, eB
                                pv = step - D
                                if 0 <= pv < kmax:
                                    toff = wid_l[pv]
                                    vbase = pv * 520
                                    nc.tensor.matmul(
                                        out=oA[:, toff:512],
                                        lhsT=Vt[:, vbase + 130 * g: vbase + 130 * g + 65],
                                        rhs=ea_l[pv][:, toff:512],
                                        start=(pv == 0), stop=(pv == kmax - 1),
                                    )
                                    nc.tensor.matmul(
                                        out=oB[:, toff:512],
                                        lhsT=Vt[:, vbase + 130 * g + 65: vbase + 130 * g + 130],
                                        rhs=eb_l[pv][:, toff:512],
                                        start=(pv == 0), stop=(pv == kmax - 1),
                                    )
                            # epilogue: normalize by rowsum (row 64), write Y^T
                            rA = rp.tile([1, 512], F32, name="rA", tag="rA")
                            rB = rp.tile([1, 512], F32, name="rB", tag="rB")
                            nc.vector.reciprocal(rA[:], oA[64:65, :])
                            nc.vector.reciprocal(rB[:], oB[64:65, :])
                            rbA = rp.tile([64, 512], F32, name="rbA", tag="rbA")
                            rbB = rp.tile([64, 512], F32, name="rbB", tag="rbB")
                            nc.gpsimd.partition_broadcast(rbA[:], rA[:])
                            nc.gpsimd.partition_broadcast(rbB[:], rB[:])
                            nc.vector.tensor_mul(
                                YT[0:64, gq + q0: gq + q0 + 512], oA[0:64, :], rbA[:])
                            nc.vector.tensor_mul(
                                YT[64:128, gq + q0: gq + q0 + 512], oB[0:64, :], rbB[:])

                        # projection for this q-chunk (runs in ACT's shadow)
                        for tt in range(4 * qc, 4 * qc + 4):
                            for oc in range(2):
                                po = ps3.tile([128, 512], F32, name="po", tag="po")
                                for g in range(4):
                                    nc.tensor.matmul(
                                        out=po[:],
                                        lhsT=YT[:, g * T + tt * 128: g * T + tt * 128 + 128],
                                        rhs=Wp_sb[:, g * C + oc * 512: g * C + oc * 512 + 512],
                                        start=(g == 0), stop=(g == 3),
                                    )
                                ob = obp.tile([128, 512], F32, name="ob", tag="ob")
                                nc.vector.tensor_copy(ob[:], po[:])
                                nc.sync.dma_start(
                                    out=out_d[tt * 128:(tt + 1) * 128,
                                              oc * 512:(oc + 1) * 512],
                                    in_=ob[:],
                                )

    nc.compile()
    return nc


def _in_maps(x, Wk, bk, Wq, bq, Wv, bv, Wp, bp, padding_mask):
    maps = []
    mask_cols = np.arange(896)[None, :]
    mask_rows = np.arange(128)[:, None]
    maskneg = np.where(mask_rows <= mask_cols - 384, 0.0, NEG).astype(np.float32)
    for core in range(8):
        b, half = divmod(core, 2)
        hs = slice(half * IC, (half + 1) * IC)
        maps.append({
            "xT": np.ascontiguousarray(x[b].T),
            "WqT": np.ascontiguousarray(Wq[hs, :].T),
            "WkT": np.ascontiguousarray(Wk[hs, :].T),
            "WvT": np.ascontiguousarray(Wv[hs, :].T),
            "WpT": np.ascontiguousarray(Wp[:, hs].T),
            "bqs": np.ascontiguousarray((bq[hs] * SCALE).reshape(4, 128).T),
            "bks": np.ascontiguousarray(bk[hs].reshape(4, 128).T),
            "bvr": bv[hs].reshape(1, IC).copy(),
            "padb": np.ascontiguousarray(
                np.where(padding_mask[b] != 0, 0.0, NEG)
                .astype(np.float32).reshape(NKT, 128).T),
            "maskneg": maskneg,
            "ones128": np.ones((1, 128), np.float32),
            "ones8": np.ones((128, 8), np.float32),
        })
    return maps


def _run(inputs, trace=False, **kw):
    if "nc" not in _CACHE:
        _CACHE["nc"] = _build()
    nc = _CACHE["nc"]
    ins = {k: np.asarray(v, dtype=np.float32) if k != "padding_mask"
           else np.asarray(v) for k, v in inputs.items()}
    maps = _in_maps(**ins)
    res = run_bass_kernel_spmd(nc, maps, core_ids=list(range(8)), trace=trace, **kw)
    bp = np.asarray(inputs["bp"], np.float32)
    y = np.empty((B, T, C), np.float32)
    for b in range(B):
        y[b] = res.results[2 * b]["out"] + res.results[2 * b + 1]["out"] + bp
    return y, res


def kernel(**inputs):
    y, _ = _run(inputs, trace=False)
    return y


# revision 9
# speedup vs baseline: 1.3572x; 1.2396x over previous
"""Causal self-attention TRN2 Bass kernel.

Problem: B=4, T=2048, C=1024, H=16 heads (HD=64), torch-Linear semantics
(y = x @ W.T + b), causal + padding mask, softmax, output projection.

Sharding: 8 cores = (batch b in 0..3) x (head-half in 0..1). Each core
handles one batch and 8 heads (512 of the 1024 channels of QKV / of the
contraction dim of the output projection). The two half-cores of a batch
produce partial output projections that the host sums (plus bp).

Per-core kernel (all matmuls in float32r — full PE rate, ~1.5e-4 rel):
  Phase 1: QKV projections.
    Q^T, K^T computed head-major ([outch, T]) so attention needs no
    transposes; V computed token-major ([T, outch]) with an interleaved
    ones column per head (rowsum trick). Attention scale (1/8) and bias
    are folded in during the PSUM->SBUF copy on DVE; V bias is added via
    a K=1 ones-outer-product matmul into PSUM.
  Phase 2: flash-style causal attention per head-pair g (2 heads packed
    on PE rows 0-63 / 64-127 via tile_position for the K=64 S^T matmuls).
    S^T[k,q] = K Q^T tiles; causal masking adds -1e30 to PSUM on the
    diagonal tiles; exp on ACT (no max subtraction needed: |S|<~3);
    O_unnorm^T[d,q] plus rowsum row via [V | 1] stationary; normalization
    via reciprocal + K=1 broadcast matmul + DVE multiply into Y^T.
  Phase 3: output projection from Y^T tiles (stationary) vs Wp^T slices.
"""

import numpy as np

import concourse.mybir as mybir
import concourse.tile as tile
from concourse import bacc
from concourse.bass_utils import run_bass_kernel_spmd

F32 = mybir.dt.float32
F32R = mybir.dt.float32r
AF = mybir.ActivationFunctionType
ALU = mybir.AluOpType

B, T, C, H = 4, 2048, 1024, 16
HD = C // H          # 64
IC = C // 2          # 512 channels per core (8 heads)
NKT = T // 128       # 16 k-tiles
NQC = T // 512       # 4 q-chunks
NCT = C // 128       # 8 contraction tiles for QKV
NEG = -1.0e30
SCALE = 1.0 / np.sqrt(HD)

_CACHE = {}


def _build():
    nc = bacc.Bacc("TRN2", target_bir_lowering=False, debug=False)

    xT_d = nc.dram_tensor("xT", [C, T], F32, kind="ExternalInput").ap()
    WqT_d = nc.dram_tensor("WqT", [C, IC], F32, kind="ExternalInput").ap()
    WkT_d = nc.dram_tensor("WkT", [C, IC], F32, kind="ExternalInput").ap()
    WvT_d = nc.dram_tensor("WvT", [C, IC], F32, kind="ExternalInput").ap()
    WpT_d = nc.dram_tensor("WpT", [IC, C], F32, kind="ExternalInput").ap()
    bq_d = nc.dram_tensor("bqs", [128, 4], F32, kind="ExternalInput").ap()
    bk_d = nc.dram_tensor("bks", [128, 4], F32, kind="ExternalInput").ap()
    bv_d = nc.dram_tensor("bvr", [1, IC], F32, kind="ExternalInput").ap()
    pad_d = nc.dram_tensor("padb", [128, NKT], F32, kind="ExternalInput").ap()
    mask_d = nc.dram_tensor("maskneg", [128, 896], F32, kind="ExternalInput").ap()
    ones128_d = nc.dram_tensor("ones128", [1, 128], F32, kind="ExternalInput").ap()
    ones8_d = nc.dram_tensor("ones8", [128, 8], F32, kind="ExternalInput").ap()
    out_d = nc.dram_tensor("out", [T, C], F32, kind="ExternalOutput").ap()

    with tile.TileContext(nc) as tc:
        with tc.tile_pool(name="pp", bufs=1) as pp:
            # Persistent SBUF state
            QT = pp.tile([128, 4 * T], F32R, name="QT")     # 4 head-pair tiles
            KT = pp.tile([128, 4 * T], F32R, name="KT")
            Vt = pp.tile([128, NKT * 520], F32R, name="Vt")  # [V|1] x 8 heads
            bq_sb = pp.tile([128, 4], F32, name="bq_sb")
            bk_sb = pp.tile([128, 4], F32, name="bk_sb")
            bv_sb = pp.tile([1, IC], F32R, name="bv_sb")
            pad_sb = pp.tile([128, NKT], F32, name="pad_sb")
            ones128 = pp.tile([1, 128], F32R, name="ones128")
            nc.sync.dma_start(out=bq_sb[:], in_=bq_d)
            nc.sync.dma_start(out=bk_sb[:], in_=bk_d)
            nc.sync.dma_start(out=bv_sb[:], in_=bv_d.bitcast(F32R))
            nc.sync.dma_start(out=pad_sb[:], in_=pad_d)
            nc.sync.dma_start(out=ones128[:], in_=ones128_d.bitcast(F32R))
            # V ones columns (col 64 of each head block of width 65)
            Vr = Vt.rearrange("p (k h c) -> p k h c", k=NKT, h=8, c=65)
            for kt in range(NKT):
                nc.sync.dma_start(out=Vr[:, kt, :, 64], in_=ones8_d.bitcast(F32R))

            # ---------------- Phase 1: QKV projections ----------------
            with tc.tile_pool(name="p1", bufs=1) as p1, \
                 tc.tile_pool(name="xs", bufs=2) as xs, \
                 tc.tile_pool(name="ps1", bufs=3, space="PSUM") as ps1:
                Wq_sb = p1.tile([128, NCT * 512], F32R, name="Wq_sb")
                Wk_sb = p1.tile([128, NCT * 512], F32R, name="Wk_sb")
                Wv_sb = p1.tile([128, NCT * 512], F32R, name="Wv_sb")
                for ct in range(NCT):
                    cs = slice(ct * 128, (ct + 1) * 128)
                    fs = slice(ct * 512, (ct + 1) * 512)
                    nc.sync.dma_start(out=Wq_sb[:, fs], in_=WqT_d[cs, :].bitcast(F32R))
                    nc.sync.dma_start(out=Wk_sb[:, fs], in_=WkT_d[cs, :].bitcast(F32R))
                    nc.sync.dma_start(out=Wv_sb[:, fs], in_=WvT_d[cs, :].bitcast(F32R))

                for tch in range(4):  # T chunks of 512
                    t0 = tch * 512
                    xc = xs.tile([128, NCT * 512], F32R, name="xc", tag="xc")
                    for ct in range(NCT):
                        nc.sync.dma_start(
                            out=xc[:, ct * 512:(ct + 1) * 512],
                            in_=xT_d[ct * 128:(ct + 1) * 128, t0:t0 + 512].bitcast(F32R),
                        )
                    # Q^T and K^T: out [outch-tile(g) 128, tokens 512]
                    for g in range(4):
                        pq = ps1.tile([128, 512], F32, name="pq", tag="ps1")
                        for ct in range(NCT):
                            nc.tensor.matmul(
                                out=pq[:],
                                lhsT=Wq_sb[:, ct * 512 + g * 128: ct * 512 + (g + 1) * 128],
                                rhs=xc[:, ct * 512:(ct + 1) * 512],
                                start=(ct == 0), stop=(ct == NCT - 1),
                            )
                        nc.vector.tensor_scalar(
                            out=QT[:, g * T + t0: g * T + t0 + 512], in0=pq[:],
                            scalar1=SCALE, scalar2=bq_sb[:, g:g + 1],
                            op0=ALU.mult, op1=ALU.add,
                        )
                        pk = ps1.tile([128, 512], F32, name="pk", tag="ps1")
                        for ct in range(NCT):
                            nc.tensor.matmul(
                                out=pk[:],
                                lhsT=Wk_sb[:, ct * 512 + g * 128: ct * 512 + (g + 1) * 128],
                                rhs=xc[:, ct * 512:(ct + 1) * 512],
                                start=(ct == 0), stop=(ct == NCT - 1),
                            )
                        nc.vector.tensor_scalar(
                            out=KT[:, g * T + t0: g * T + t0 + 512], in0=pk[:],
                            scalar1=bk_sb[:, g:g + 1], scalar2=None, op0=ALU.add,
                        )
                    # V: out [token-tile 128, outch 512] (+ bias via K=1 matmul)
                    for ts in range(4):
                        kt = tch * 4 + ts
                        pv = ps1.tile([128, 512], F32, name="pv", tag="ps1")
                        for ct in range(NCT):
                            nc.tensor.matmul(
                                out=pv[:],
                                lhsT=xc[:, ct * 512 + ts * 128: ct * 512 + ts * 128 + 128],
                                rhs=Wv_sb[:, ct * 512:(ct + 1) * 512],
                                start=(ct == 0), stop=False,
                            )
                        nc.tensor.matmul(
                            out=pv[:], lhsT=ones128[:], rhs=bv_sb[:],
                            start=False, stop=True,
                        )
                        nc.vector.tensor_copy(Vr[:, kt, :, 0:64], pv[:])

            # ---------------- Phase 2: causal attention + projection ----------
            # qc-outer / g-inner; S and PV interleaved with skew D; S-pair
            # PSUM packed into one 2-bank tile so each kt needs ONE exp
            # (3D AP over both heads); diagonal tiles trimmed; projection
            # matmul groups for the previous q-chunk are spread through the
            # attention stream as PE filler (keeps the HAM clock-gate warm).
            D = 4
            with tc.tile_pool(name="p23", bufs=1) as p23:
                YT = p23.tile([128, 4 * T], F32R, name="YT")
                mask_sb = p23.tile([128, 896], F32, name="mask_sb")
                Wp_sb = p23.tile([128, 4 * C], F32R, name="Wp_sb")
                nc.sync.dma_start(out=mask_sb[:], in_=mask_d)
                for g in range(4):
                    nc.sync.dma_start(
                        out=Wp_sb[:, g * C:(g + 1) * C],
                        in_=WpT_d[g * 128:(g + 1) * 128, :].bitcast(F32R),
                    )

                with tc.tile_pool(name="es", bufs=D + 1) as es, \
                     tc.tile_pool(name="rp", bufs=2) as rp, \
                     tc.tile_pool(name="ob", bufs=3) as obp, \
                     tc.tile_pool(name="pss", bufs=2, space="PSUM") as pss, \
                     tc.tile_pool(name="pso", bufs=2, space="PSUM") as pso:

                    def proj_group(tt, oc):
                        po = pso.tile([128, 512], F32, name="po", tag="o")
                        for g in range(4):
                            nc.tensor.matmul(
                                out=po[:],
                                lhsT=YT[:, g * T + tt * 128: g * T + tt * 128 + 128],
                                rhs=Wp_sb[:, g * C + oc * 512: g * C + oc * 512 + 512],
                                start=(g == 0), stop=(g == 3),
                            )
                        ob = obp.tile([128, 512], F32, name="ob", tag="ob")
                        nc.vector.tensor_copy(ob[:], po[:])
                        nc.sync.dma_start(
                            out=out_d[tt * 128:(tt + 1) * 128,
                                      oc * 512:(oc + 1) * 512],
                            in_=ob[:],
                        )

                    for qc in range(NQC):
                        q0 = qc * 512
                        kmax = 4 * qc + 4
                        # proj groups of the previous q-chunk, spread over g's
                        pending = ([(tt, oc) for tt in range(4 * qc - 4, 4 * qc)
                                    for oc in range(2)] if qc > 0 else [])
                        for g in range(4):
                            gq = g * T
                            oAB = pso.tile([65, 1024], F32, name="oAB", tag="o")
                            e_l = [None] * kmax
                            off_l = [None] * kmax
                            for step in range(kmax + D):
                                if step < kmax:
                                    kt = step
                                    k0 = kt * 128
                                    toff = 128 * (kt - 4 * qc) if kt >= 4 * qc else 0
                                    w = 512 - toff
                                    off_l[kt] = toff
                                    sAB = pss.tile([128, 1024], F32, name="sAB", tag="sAB")
                                    nc.tensor.matmul(
                                        out=sAB[:, toff:512],
                                        lhsT=KT[0:64, gq + k0: gq + k0 + 128],
                                        rhs=QT[0:64, gq + q0 + toff: gq + q0 + 512],
                                        start=True, stop=True,
                                    )
                                    nc.tensor.matmul(
                                        out=sAB[:, 512 + toff:1024],
                                        lhsT=KT[64:128, gq + k0: gq + k0 + 128],
                                        rhs=QT[64:128, gq + q0 + toff: gq + q0 + 512],
                                        start=True, stop=True, tile_position=(64, 0),
                                    )
                                    if kt >= 4 * qc:  # diagonal: additive causal mask
                                        ms = mask_sb[:, 384:384 + w]
                                        nc.vector.tensor_add(
                                            sAB[:, toff:512], sAB[:, toff:512], ms)
                                        nc.vector.tensor_add(
                                            sAB[:, 512 + toff:1024],
                                            sAB[:, 512 + toff:1024], ms)
                                    eAB = es.tile([128, 1024], F32R, name="eAB", tag="eAB")
                                    s3 = sAB.rearrange("p (h w) -> p h w", h=2, w=512)
                                    e3 = eAB.rearrange("p (h w) -> p h w", h=2, w=512)
                                    nc.scalar.activation(
                                        e3[:, :, toff:512], s3[:, :, toff:512], AF.Exp,
                                        bias=pad_sb[:, kt:kt + 1])
                                    e_l[kt] = eAB
                                pv = step - D
                                if 0 <= pv < kmax:
                                    toff = off_l[pv]
                                    vbase = pv * 520
                                    nc.tensor.matmul(
                                        out=oAB[:, toff:512],
                                        lhsT=Vt[:, vbase + 130 * g: vbase + 130 * g + 65],
                                        rhs=e_l[pv][:, toff:512],
                                        start=(pv == 0), stop=(pv == kmax - 1),
                                    )
                                    nc.tensor.matmul(
                                        out=oAB[:, 512 + toff:1024],
                                        lhsT=Vt[:, vbase + 130 * g + 65: vbase + 130 * g + 130],
                                        rhs=e_l[pv][:, 512 + toff:1024],
                                        start=(pv == 0), stop=(pv == kmax - 1),
                                    )
                            # epilogue: normalize by rowsum (row 64), write Y^T
                            rA = rp.tile([1, 512], F32, name="rA", tag="rA")
                            rB = rp.tile([1, 512], F32, name="rB", tag="rB")
                            nc.vector.reciprocal(rA[:], oAB[64:65, 0:512])
                            nc.vector.reciprocal(rB[:], oAB[64:65, 512:1024])
                            rbA = rp.tile([64, 512], F32, name="rbA", tag="rbA")
                            rbB = rp.tile([64, 512], F32, name="rbB", tag="rbB")
                            nc.gpsimd.partition_broadcast(rbA[:], rA[:])
                            nc.gpsimd.partition_broadcast(rbB[:], rB[:])
                            nc.vector.tensor_mul(
                                YT[0:64, gq + q0: gq + q0 + 512],
                                oAB[0:64, 0:512], rbA[:])
                            nc.vector.tensor_mul(
                                YT[64:128, gq + q0: gq + q0 + 512],
                                oAB[0:64, 512:1024], rbB[:])
                            # PE filler: two proj groups of the previous chunk
                            for _ in range(2):
                                if pending:
                                    proj_group(*pending.pop(0))
                    # final chunk's projection
                    for tt in range(12, 16):
                        for oc in range(2):
                            proj_group(tt, oc)

    nc.compile()
    return nc


def _in_maps(x, Wk, bk, Wq, bq, Wv, bv, Wp, bp, padding_mask):
    maps = []
    mask_cols = np.arange(896)[None, :]
    mask_rows = np.arange(128)[:, None]
    maskneg = np.where(mask_rows <= mask_cols - 384, 0.0, NEG).astype(np.float32)
    for core in range(8):
        b, half = divmod(core, 2)
        hs = slice(half * IC, (half + 1) * IC)
        maps.append({
            "xT": np.ascontiguousarray(x[b].T),
            "WqT": np.ascontiguousarray(Wq[hs, :].T),
            "WkT": np.ascontiguousarray(Wk[hs, :].T),
            "WvT": np.ascontiguousarray(Wv[hs, :].T),
            "WpT": np.ascontiguousarray(Wp[:, hs].T),
            "bqs": np.ascontiguousarray((bq[hs] * SCALE).reshape(4, 128).T),
            "bks": np.ascontiguousarray(bk[hs].reshape(4, 128).T),
            "bvr": bv[hs].reshape(1, IC).copy(),
            "padb": np.ascontiguousarray(
                np.where(padding_mask[b] != 0, 0.0, NEG)
                .astype(np.float32).reshape(NKT, 128).T),
            "maskneg": maskneg,
            "ones128": np.ones((1, 128), np.float32),
            "ones8": np.ones((128, 8), np.float32),
        })
    return maps


def _run(inputs, trace=False, **kw):
    if "nc" not in _CACHE:
        _CACHE["nc"] = _build()
    nc = _CACHE["nc"]
    ins = {k: np.asarray(v, dtype=np.float32) if k != "padding_mask"
           else np.asarray(v) for k, v in inputs.items()}
    maps = _in_maps(**ins)
    res = run_bass_kernel_spmd(nc, maps, core_ids=list(range(8)), trace=trace, **kw)
    bp = np.asarray(inputs["bp"], np.float32)
    y = np.empty((B, T, C), np.float32)
    for b in range(B):
        y[b] = res.results[2 * b]["out"] + res.results[2 * b + 1]["out"] + bp
    return y, res


def kernel(**inputs):
    y, _ = _run(inputs, trace=False)
    return y


# revision 10
# speedup vs baseline: 1.4356x; 1.0577x over previous
"""Causal self-attention TRN2 Bass kernel.

Problem: B=4, T=2048, C=1024, H=16 heads (HD=64), torch-Linear semantics
(y = x @ W.T + b), causal + padding mask, softmax, output projection.

Sharding: 8 cores = (batch b in 0..3) x (head-half in 0..1). Each core
handles one batch and 8 heads (512 of the 1024 channels of QKV / of the
contraction dim of the output projection). The two half-cores of a batch
produce partial output projections that the host sums (plus bp).

Per-core kernel (all matmuls in float32r — full PE rate, ~1.5e-4 rel):
  Phase 1: QKV projections.
    Q^T, K^T computed head-major ([outch, T]) so attention needs no
    transposes; V computed token-major ([T, outch]) with an interleaved
    ones column per head (rowsum trick). Attention scale (1/8) and bias
    are folded in during the PSUM->SBUF copy on DVE; V bias is added via
    a K=1 ones-outer-product matmul into PSUM.
  Phase 2: flash-style causal attention per head-pair g (2 heads packed
    on PE rows 0-63 / 64-127 via tile_position for the K=64 S^T matmuls).
    S^T[k,q] = K Q^T tiles; causal masking adds -1e30 to PSUM on the
    diagonal tiles; exp on ACT (no max subtraction needed: |S|<~3);
    O_unnorm^T[d,q] plus rowsum row via [V | 1] stationary; normalization
    via reciprocal + K=1 broadcast matmul + DVE multiply into Y^T.
  Phase 3: output projection from Y^T tiles (stationary) vs Wp^T slices.
"""

import numpy as np

import concourse.mybir as mybir
import concourse.tile as tile
from concourse import bacc
from concourse.bass_utils import run_bass_kernel_spmd

F32 = mybir.dt.float32
F32R = mybir.dt.float32r
AF = mybir.ActivationFunctionType
ALU = mybir.AluOpType

B, T, C, H = 4, 2048, 1024, 16
HD = C // H          # 64
IC = C // 2          # 512 channels per core (8 heads)
NKT = T // 128       # 16 k-tiles
NQC = T // 512       # 4 q-chunks
NCT = C // 128       # 8 contraction tiles for QKV
NEG = -1.0e30
SCALE = 1.0 / np.sqrt(HD)

_CACHE = {}


def _build():
    nc = bacc.Bacc("TRN2", target_bir_lowering=False, debug=False)

    xT_d = nc.dram_tensor("xT", [C, T], F32, kind="ExternalInput").ap()
    WqT_d = nc.dram_tensor("WqT", [C, IC], F32, kind="ExternalInput").ap()
    WkT_d = nc.dram_tensor("WkT", [C, IC], F32, kind="ExternalInput").ap()
    WvT_d = nc.dram_tensor("WvT", [C, IC], F32, kind="ExternalInput").ap()
    WpT_d = nc.dram_tensor("WpT", [IC, C], F32, kind="ExternalInput").ap()
    bq_d = nc.dram_tensor("bqs", [128, 4], F32, kind="ExternalInput").ap()
    bk_d = nc.dram_tensor("bks", [128, 4], F32, kind="ExternalInput").ap()
    bv_d = nc.dram_tensor("bvr", [1, IC], F32, kind="ExternalInput").ap()
    pad_d = nc.dram_tensor("padb", [128, NKT], F32, kind="ExternalInput").ap()
    mask_d = nc.dram_tensor("maskneg", [128, 896], F32, kind="ExternalInput").ap()
    ones128_d = nc.dram_tensor("ones128", [1, 128], F32, kind="ExternalInput").ap()
    ones8_d = nc.dram_tensor("ones8", [128, 8], F32, kind="ExternalInput").ap()
    ident_d = nc.dram_tensor("ident", [128, 128], F32, kind="ExternalInput").ap()
    out_d = nc.dram_tensor("out", [T, C], F32, kind="ExternalOutput").ap()

    with tile.TileContext(nc) as tc:
        with tc.tile_pool(name="pp", bufs=1) as pp:
            # Persistent SBUF state
            QT = pp.tile([128, 4 * T], F32R, name="QT")     # 4 head-pair tiles
            KT = pp.tile([128, 4 * T], F32R, name="KT")
            Vt = pp.tile([128, NKT * 520], F32R, name="Vt")  # [V|1] x 8 heads
            bq_sb = pp.tile([128, 4], F32, name="bq_sb")
            bk_sb = pp.tile([128, 4], F32, name="bk_sb")
            bv_sb = pp.tile([1, IC], F32R, name="bv_sb")
            pad_sb = pp.tile([128, NKT], F32, name="pad_sb")
            ones128 = pp.tile([1, 128], F32R, name="ones128")
            nc.sync.dma_start(out=bq_sb[:], in_=bq_d)
            nc.sync.dma_start(out=bk_sb[:], in_=bk_d)
            nc.sync.dma_start(out=bv_sb[:], in_=bv_d.bitcast(F32R))
            nc.sync.dma_start(out=pad_sb[:], in_=pad_d)
            nc.sync.dma_start(out=ones128[:], in_=ones128_d.bitcast(F32R))
            # V ones columns (col 64 of each head block of width 65)
            Vr = Vt.rearrange("p (k h c) -> p k h c", k=NKT, h=8, c=65)
            for kt in range(NKT):
                nc.sync.dma_start(out=Vr[:, kt, :, 64], in_=ones8_d.bitcast(F32R))

            # ---------------- Phase 1: QKV projections ----------------
            with tc.tile_pool(name="p1", bufs=1) as p1, \
                 tc.tile_pool(name="xs", bufs=2) as xs, \
                 tc.tile_pool(name="ps1", bufs=3, space="PSUM") as ps1:
                Wq_sb = p1.tile([128, NCT * 512], F32R, name="Wq_sb")
                Wk_sb = p1.tile([128, NCT * 512], F32R, name="Wk_sb")
                Wv_sb = p1.tile([128, NCT * 512], F32R, name="Wv_sb")
                for ct in range(NCT):
                    cs = slice(ct * 128, (ct + 1) * 128)
                    fs = slice(ct * 512, (ct + 1) * 512)
                    nc.sync.dma_start(out=Wq_sb[:, fs], in_=WqT_d[cs, :].bitcast(F32R))
                    nc.sync.dma_start(out=Wk_sb[:, fs], in_=WkT_d[cs, :].bitcast(F32R))
                    nc.sync.dma_start(out=Wv_sb[:, fs], in_=WvT_d[cs, :].bitcast(F32R))

                for tch in range(4):  # T chunks of 512
                    t0 = tch * 512
                    xc = xs.tile([128, NCT * 512], F32R, name="xc", tag="xc")
                    for ct in range(NCT):
                        nc.sync.dma_start(
                            out=xc[:, ct * 512:(ct + 1) * 512],
                            in_=xT_d[ct * 128:(ct + 1) * 128, t0:t0 + 512].bitcast(F32R),
                        )
                    # Q^T and K^T: out [outch-tile(g) 128, tokens 512]
                    for g in range(4):
                        pq = ps1.tile([128, 512], F32, name="pq", tag="ps1")
                        for ct in range(NCT):
                            nc.tensor.matmul(
                                out=pq[:],
                                lhsT=Wq_sb[:, ct * 512 + g * 128: ct * 512 + (g + 1) * 128],
                                rhs=xc[:, ct * 512:(ct + 1) * 512],
                                start=(ct == 0), stop=(ct == NCT - 1),
                            )
                        nc.vector.tensor_scalar(
                            out=QT[:, g * T + t0: g * T + t0 + 512], in0=pq[:],
                            scalar1=SCALE, scalar2=bq_sb[:, g:g + 1],
                            op0=ALU.mult, op1=ALU.add,
                        )
                        pk = ps1.tile([128, 512], F32, name="pk", tag="ps1")
                        for ct in range(NCT):
                            nc.tensor.matmul(
                                out=pk[:],
                                lhsT=Wk_sb[:, ct * 512 + g * 128: ct * 512 + (g + 1) * 128],
                                rhs=xc[:, ct * 512:(ct + 1) * 512],
                                start=(ct == 0), stop=(ct == NCT - 1),
                            )
                        nc.vector.tensor_scalar(
                            out=KT[:, g * T + t0: g * T + t0 + 512], in0=pk[:],
                            scalar1=bk_sb[:, g:g + 1], scalar2=None, op0=ALU.add,
                        )
                    # V: out [token-tile 128, outch 512] (+ bias via K=1 matmul)
                    for ts in range(4):
                        kt = tch * 4 + ts
                        pv = ps1.tile([128, 512], F32, name="pv", tag="ps1")
                        for ct in range(NCT):
                            nc.tensor.matmul(
                                out=pv[:],
                                lhsT=xc[:, ct * 512 + ts * 128: ct * 512 + ts * 128 + 128],
                                rhs=Wv_sb[:, ct * 512:(ct + 1) * 512],
                                start=(ct == 0), stop=False,
                            )
                        nc.tensor.matmul(
                            out=pv[:], lhsT=ones128[:], rhs=bv_sb[:],
                            start=False, stop=True,
                        )
                        nc.vector.tensor_copy(Vr[:, kt, :, 0:64], pv[:])

            # ---------------- Phase 2: causal attention + projection ----------
            # qc-outer / g-inner; S and PV interleaved with skew D; S-pair
            # PSUM packed into one 2-bank tile so each kt needs ONE exp
            # (3D AP over both heads); diagonal tiles trimmed; projection
            # matmul groups for the previous q-chunk are spread through the
            # attention stream as PE filler (keeps the HAM clock-gate warm).
            D = 4
            with tc.tile_pool(name="p23", bufs=1) as p23:
                YT = p23.tile([128, 4 * T], F32R, name="YT")
                mask_sb = p23.tile([128, 128], F32R, name="mask_sb")
                ident_sb = p23.tile([128, 128], F32R, name="ident_sb")
                Wp_sb = p23.tile([128, 4 * C], F32R, name="Wp_sb")
                nc.sync.dma_start(out=mask_sb[:], in_=mask_d[:, 384:512].bitcast(F32R))
                nc.sync.dma_start(out=ident_sb[:], in_=ident_d.bitcast(F32R))
                for g in range(4):
                    nc.sync.dma_start(
                        out=Wp_sb[:, g * C:(g + 1) * C],
                        in_=WpT_d[g * 128:(g + 1) * 128, :].bitcast(F32R),
                    )

                with tc.tile_pool(name="es", bufs=D + 1) as es, \
                     tc.tile_pool(name="rp", bufs=2) as rp, \
                     tc.tile_pool(name="ob", bufs=3) as obp, \
                     tc.tile_pool(name="pss", bufs=2, space="PSUM") as pss, \
                     tc.tile_pool(name="pso", bufs=2, space="PSUM") as pso:

                    def proj_group(tt, oc):
                        po = pso.tile([128, 512], F32, name="po", tag="o")
                        for g in range(4):
                            nc.tensor.matmul(
                                out=po[:],
                                lhsT=YT[:, g * T + tt * 128: g * T + tt * 128 + 128],
                                rhs=Wp_sb[:, g * C + oc * 512: g * C + oc * 512 + 512],
                                start=(g == 0), stop=(g == 3),
                            )
                        ob = obp.tile([128, 512], F32, name="ob", tag="ob")
                        nc.vector.tensor_copy(ob[:], po[:])
                        nc.sync.dma_start(
                            out=out_d[tt * 128:(tt + 1) * 128,
                                      oc * 512:(oc + 1) * 512],
                            in_=ob[:],
                        )

                    prev_qc = None
                    for qc in (3, 2, 1, 0):
                        q0 = qc * 512
                        kmax = 4 * qc + 4
                        # proj groups of the previously finished q-chunk
                        pending = ([(tt, oc) for tt in range(4 * prev_qc, 4 * prev_qc + 4)
                                    for oc in range(2)] if prev_qc is not None else [])
                        for g in range(4):
                            gq = g * T
                            oAB = pso.tile([65, 1024], F32, name="oAB", tag="o")
                            e_l = [None] * kmax
                            off_l = [None] * kmax
                            for step in range(kmax + D):
                                if step < kmax:
                                    kt = step
                                    k0 = kt * 128
                                    toff = 128 * (kt - 4 * qc) if kt >= 4 * qc else 0
                                    w = 512 - toff
                                    off_l[kt] = toff
                                    diag = kt >= 4 * qc
                                    sAB = pss.tile([128, 1024], F32, name="sAB", tag="sAB")
                                    nc.tensor.matmul(
                                        out=sAB[:, toff:512],
                                        lhsT=KT[0:64, gq + k0: gq + k0 + 128],
                                        rhs=QT[0:64, gq + q0 + toff: gq + q0 + 512],
                                        start=True, stop=not diag,
                                    )
                                    nc.tensor.matmul(
                                        out=sAB[:, 512 + toff:1024],
                                        lhsT=KT[64:128, gq + k0: gq + k0 + 128],
                                        rhs=QT[64:128, gq + q0 + toff: gq + q0 + 512],
                                        start=True, stop=not diag, tile_position=(64, 0),
                                    )
                                    if diag:
                                        # additive causal mask on the 128-wide
                                        # diagonal band, via identity matmul
                                        nc.tensor.matmul(
                                            out=sAB[:, toff:toff + 128],
                                            lhsT=ident_sb[:],
                                            rhs=mask_sb[:],
                                            start=False, stop=True,
                                        )
                                        nc.tensor.matmul(
                                            out=sAB[:, 512 + toff:512 + toff + 128],
                                            lhsT=ident_sb[:],
                                            rhs=mask_sb[:],
                                            start=False, stop=True,
                                        )
                                    eAB = es.tile([128, 1024], F32R, name="eAB", tag="eAB")
                                    s3 = sAB.rearrange("p (h w) -> p h w", h=2, w=512)
                                    e3 = eAB.rearrange("p (h w) -> p h w", h=2, w=512)
                                    nc.scalar.activation(
                                        e3[:, :, toff:512], s3[:, :, toff:512], AF.Exp,
                                        bias=pad_sb[:, kt:kt + 1])
                                    e_l[kt] = eAB
                                pv = step - D
                                if 0 <= pv < kmax:
                                    toff = off_l[pv]
                                    vbase = pv * 520
                                    nc.tensor.matmul(
                                        out=oAB[:, toff:512],
                                        lhsT=Vt[:, vbase + 130 * g: vbase + 130 * g + 65],
                                        rhs=e_l[pv][:, toff:512],
                                        start=(pv == 0), stop=(pv == kmax - 1),
                                    )
                                    nc.tensor.matmul(
                                        out=oAB[:, 512 + toff:1024],
                                        lhsT=Vt[:, vbase + 130 * g + 65: vbase + 130 * g + 130],
                                        rhs=e_l[pv][:, 512 + toff:1024],
                                        start=(pv == 0), stop=(pv == kmax - 1),
                                    )
                            # epilogue: normalize by rowsum (row 64), write Y^T
                            rA = rp.tile([1, 512], F32, name="rA", tag="rA")
                            rB = rp.tile([1, 512], F32, name="rB", tag="rB")
                            nc.vector.reciprocal(rA[:], oAB[64:65, 0:512])
                            nc.vector.reciprocal(rB[:], oAB[64:65, 512:1024])
                            rbA = rp.tile([64, 512], F32, name="rbA", tag="rbA")
                            rbB = rp.tile([64, 512], F32, name="rbB", tag="rbB")
                            nc.gpsimd.partition_broadcast(rbA[:], rA[:])
                            nc.gpsimd.partition_broadcast(rbB[:], rB[:])
                            nc.vector.tensor_mul(
                                YT[0:64, gq + q0: gq + q0 + 512],
                                oAB[0:64, 0:512], rbA[:])
                            nc.vector.tensor_mul(
                                YT[64:128, gq + q0: gq + q0 + 512],
                                oAB[0:64, 512:1024], rbB[:])
                            # PE filler: two proj groups of the previous chunk
                            for _ in range(2):
                                if pending:
                                    proj_group(*pending.pop(0))
                        prev_qc = qc
                    # last processed chunk's projection
                    for tt in range(0, 4):
                        for oc in range(2):
                            proj_group(tt, oc)

    nc.compile()
    return nc


def _in_maps(x, Wk, bk, Wq, bq, Wv, bv, Wp, bp, padding_mask):
    maps = []
    mask_cols = np.arange(896)[None, :]
    mask_rows = np.arange(128)[:, None]
    maskneg = np.where(mask_rows <= mask_cols - 384, 0.0, NEG).astype(np.float32)
    for core in range(8):
        b, half = divmod(core, 2)
        hs = slice(half * IC, (half + 1) * IC)
        maps.append({
            "xT": np.ascontiguousarray(x[b].T),
            "WqT": np.ascontiguousarray(Wq[hs, :].T),
            "WkT": np.ascontiguousarray(Wk[hs, :].T),
            "WvT": np.ascontiguousarray(Wv[hs, :].T),
            "WpT": np.ascontiguousarray(Wp[:, hs].T),
            "bqs": np.ascontiguousarray((bq[hs] * SCALE).reshape(4, 128).T),
            "bks": np.ascontiguousarray(bk[hs].reshape(4, 128).T),
            "bvr": bv[hs].reshape(1, IC).copy(),
            "padb": np.ascontiguousarray(
                np.where(padding_mask[b] != 0, 0.0, NEG)
                .astype(np.float32).reshape(NKT, 128).T),
            "maskneg": maskneg,
            "ones128": np.ones((1, 128), np.float32),
            "ones8": np.ones((128, 8), np.float32),
            "ident": np.eye(128, dtype=np.float32),
        })
    return maps


def _run(inputs, trace=False, **kw):
    if "nc" not in _CACHE:
        _CACHE["nc"] = _build()
    nc = _CACHE["nc"]
    ins = {k: np.asarray(v, dtype=np.float32) if k != "padding_mask"
           else np.asarray(v) for k, v in inputs.items()}
    maps = _in_maps(**ins)
    res = run_bass_kernel_spmd(nc, maps, core_ids=list(range(8)), trace=trace, **kw)
    bp = np.asarray(inputs["bp"], np.float32)
    y = np.empty((B, T, C), np.float32)
    for b in range(B):
        y[b] = res.results[2 * b]["out"] + res.results[2 * b + 1]["out"] + bp
    return y, res


def kernel(**inputs):
    y, _ = _run(inputs, trace=False)
    return y


# revision 11
# speedup vs baseline: 1.5196x; 1.0585x over previous
"""Causal self-attention TRN2 Bass kernel.

Problem: B=4, T=2048, C=1024, H=16 heads (HD=64), torch-Linear semantics
(y = x @ W.T + b), causal + padding mask, softmax, output projection.

Sharding: 8 cores = (batch b in 0..3) x (head-half in 0..1). Each core
handles one batch and 8 heads (512 of the 1024 channels of QKV / of the
contraction dim of the output projection). The two half-cores of a batch
produce partial output projections that the host sums (plus bp).

Per-core kernel (all matmuls in float32r — full PE rate, ~1.5e-4 rel):
  Phase 1: QKV projections.
    Q^T, K^T computed head-major ([outch, T]) so attention needs no
    transposes; V computed token-major ([T, outch]) with an interleaved
    ones column per head (rowsum trick). Attention scale (1/8) and bias
    are folded in during the PSUM->SBUF copy on DVE; V bias is added via
    a K=1 ones-outer-product matmul into PSUM.
  Phase 2: flash-style causal attention per head-pair g (2 heads packed
    on PE rows 0-63 / 64-127 via tile_position for the K=64 S^T matmuls).
    S^T[k,q] = K Q^T tiles; causal masking adds -1e30 to PSUM on the
    diagonal tiles; exp on ACT (no max subtraction needed: |S|<~3);
    O_unnorm^T[d,q] plus rowsum row via [V | 1] stationary; normalization
    via reciprocal + K=1 broadcast matmul + DVE multiply into Y^T.
  Phase 3: output projection from Y^T tiles (stationary) vs Wp^T slices.
"""

import ml_dtypes
import numpy as np

import concourse.mybir as mybir
import concourse.tile as tile
from concourse import bacc
from concourse.bass_utils import run_bass_kernel_spmd

F32 = mybir.dt.float32
F32R = mybir.dt.float32r
BF16 = mybir.dt.bfloat16
AF = mybir.ActivationFunctionType
ALU = mybir.AluOpType

B, T, C, H = 4, 2048, 1024, 16
HD = C // H          # 64
IC = C // 2          # 512 channels per core (8 heads)
NKT = T // 128       # 16 k-tiles
NQC = T // 512       # 4 q-chunks
NCT = C // 128       # 8 contraction tiles for QKV
NEG = -1.0e30
SCALE = 1.0 / np.sqrt(HD)

_CACHE = {}


def _build():
    nc = bacc.Bacc("TRN2", target_bir_lowering=False, debug=False)

    xT_d = nc.dram_tensor("xT", [C, T], F32, kind="ExternalInput").ap()
    WqT_d = nc.dram_tensor("WqT", [C, IC], F32, kind="ExternalInput").ap()
    WkT_d = nc.dram_tensor("WkT", [C, IC], F32, kind="ExternalInput").ap()
    WvT_d = nc.dram_tensor("WvT", [C, IC], F32, kind="ExternalInput").ap()
    WpT_d = nc.dram_tensor("WpT", [IC, C], F32, kind="ExternalInput").ap()
    bq_d = nc.dram_tensor("bqs", [128, 4], F32, kind="ExternalInput").ap()
    bk_d = nc.dram_tensor("bks", [128, 4], F32, kind="ExternalInput").ap()
    bv_d = nc.dram_tensor("bvr", [1, IC], F32, kind="ExternalInput").ap()
    pad_d = nc.dram_tensor("padb", [128, NKT], F32, kind="ExternalInput").ap()
    mask_d = nc.dram_tensor("maskneg", [128, 896], F32, kind="ExternalInput").ap()
    ones128_d = nc.dram_tensor("ones128", [1, 128], F32, kind="ExternalInput").ap()
    ones8_d = nc.dram_tensor("ones8", [128, 8], BF16, kind="ExternalInput").ap()
    ident_d = nc.dram_tensor("ident", [128, 128], BF16, kind="ExternalInput").ap()
    maskb_d = nc.dram_tensor("maskb", [128, 128], BF16, kind="ExternalInput").ap()
    out_d = nc.dram_tensor("out", [T, C], F32, kind="ExternalOutput").ap()

    with tile.TileContext(nc) as tc:
        with tc.tile_pool(name="pp", bufs=1) as pp:
            # Persistent SBUF state
            QT = pp.tile([128, 4 * T], BF16, name="QT")     # 4 head-pair tiles
            KT = pp.tile([128, 4 * T], BF16, name="KT")
            Vt = pp.tile([128, NKT * 520], BF16, name="Vt")  # [V|1] x 8 heads
            bq_sb = pp.tile([128, 4], F32, name="bq_sb")
            bk_sb = pp.tile([128, 4], F32, name="bk_sb")
            bv_sb = pp.tile([1, IC], F32R, name="bv_sb")
            pad_sb = pp.tile([128, NKT], F32, name="pad_sb")
            ones128 = pp.tile([1, 128], F32R, name="ones128")
            nc.sync.dma_start(out=bq_sb[:], in_=bq_d)
            nc.sync.dma_start(out=bk_sb[:], in_=bk_d)
            nc.sync.dma_start(out=bv_sb[:], in_=bv_d.bitcast(F32R))
            nc.sync.dma_start(out=pad_sb[:], in_=pad_d)
            nc.sync.dma_start(out=ones128[:], in_=ones128_d.bitcast(F32R))
            # V ones columns (col 64 of each head block of width 65)
            Vr = Vt.rearrange("p (k h c) -> p k h c", k=NKT, h=8, c=65)
            for kt in range(NKT):
                nc.sync.dma_start(out=Vr[:, kt, :, 64], in_=ones8_d)

            # ---------------- Phase 1: QKV projections ----------------
            with tc.tile_pool(name="p1", bufs=1) as p1, \
                 tc.tile_pool(name="xs", bufs=2) as xs, \
                 tc.tile_pool(name="ps1", bufs=3, space="PSUM") as ps1:
                Wq_sb = p1.tile([128, NCT * 512], F32R, name="Wq_sb")
                Wk_sb = p1.tile([128, NCT * 512], F32R, name="Wk_sb")
                Wv_sb = p1.tile([128, NCT * 512], F32R, name="Wv_sb")
                for ct in range(NCT):
                    cs = slice(ct * 128, (ct + 1) * 128)
                    fs = slice(ct * 512, (ct + 1) * 512)
                    nc.sync.dma_start(out=Wq_sb[:, fs], in_=WqT_d[cs, :].bitcast(F32R))
                    nc.sync.dma_start(out=Wk_sb[:, fs], in_=WkT_d[cs, :].bitcast(F32R))
                    nc.sync.dma_start(out=Wv_sb[:, fs], in_=WvT_d[cs, :].bitcast(F32R))

                for tch in range(4):  # T chunks of 512
                    t0 = tch * 512
                    xc = xs.tile([128, NCT * 512], F32R, name="xc", tag="xc")
                    for ct in range(NCT):
                        nc.sync.dma_start(
                            out=xc[:, ct * 512:(ct + 1) * 512],
                            in_=xT_d[ct * 128:(ct + 1) * 128, t0:t0 + 512].bitcast(F32R),
                        )
                    # Q^T and K^T: out [outch-tile(g) 128, tokens 512]
                    for g in range(4):
                        pq = ps1.tile([128, 512], F32, name="pq", tag="ps1")
                        for ct in range(NCT):
                            nc.tensor.matmul(
                                out=pq[:],
                                lhsT=Wq_sb[:, ct * 512 + g * 128: ct * 512 + (g + 1) * 128],
                                rhs=xc[:, ct * 512:(ct + 1) * 512],
                                start=(ct == 0), stop=(ct == NCT - 1),
                            )
                        nc.vector.tensor_scalar(
                            out=QT[:, g * T + t0: g * T + t0 + 512], in0=pq[:],
                            scalar1=SCALE, scalar2=bq_sb[:, g:g + 1],
                            op0=ALU.mult, op1=ALU.add,
                        )
                        pk = ps1.tile([128, 512], F32, name="pk", tag="ps1")
                        for ct in range(NCT):
                            nc.tensor.matmul(
                                out=pk[:],
                                lhsT=Wk_sb[:, ct * 512 + g * 128: ct * 512 + (g + 1) * 128],
                                rhs=xc[:, ct * 512:(ct + 1) * 512],
                                start=(ct == 0), stop=(ct == NCT - 1),
                            )
                        nc.vector.tensor_scalar(
                            out=KT[:, g * T + t0: g * T + t0 + 512], in0=pk[:],
                            scalar1=bk_sb[:, g:g + 1], scalar2=None, op0=ALU.add,
                        )
                    # V: out [token-tile 128, outch 512] (+ bias via K=1 matmul)
                    for ts in range(4):
                        kt = tch * 4 + ts
                        pv = ps1.tile([128, 512], F32, name="pv", tag="ps1")
                        for ct in range(NCT):
                            nc.tensor.matmul(
                                out=pv[:],
                                lhsT=xc[:, ct * 512 + ts * 128: ct * 512 + ts * 128 + 128],
                                rhs=Wv_sb[:, ct * 512:(ct + 1) * 512],
                                start=(ct == 0), stop=False,
                            )
                        nc.tensor.matmul(
                            out=pv[:], lhsT=ones128[:], rhs=bv_sb[:],
                            start=False, stop=True,
                        )
                        nc.vector.tensor_copy(Vr[:, kt, :, 0:64], pv[:])

            # ---------------- Phase 2: causal attention + projection ----------
            # qc-outer / g-inner; S and PV interleaved with skew D; S-pair
            # PSUM packed into one 2-bank tile so each kt needs ONE exp
            # (3D AP over both heads); diagonal tiles trimmed; projection
            # matmul groups for the previous q-chunk are spread through the
            # attention stream as PE filler (keeps the HAM clock-gate warm).
            D = 4
            with tc.tile_pool(name="p23", bufs=1) as p23:
                YT = p23.tile([128, 4 * T], F32R, name="YT")
                mask_sb = p23.tile([128, 128], BF16, name="mask_sb")
                ident_sb = p23.tile([128, 128], BF16, name="ident_sb")
                Wp_sb = p23.tile([128, 4 * C], F32R, name="Wp_sb")
                nc.sync.dma_start(out=mask_sb[:], in_=maskb_d)
                nc.sync.dma_start(out=ident_sb[:], in_=ident_d)
                for g in range(4):
                    nc.sync.dma_start(
                        out=Wp_sb[:, g * C:(g + 1) * C],
                        in_=WpT_d[g * 128:(g + 1) * 128, :].bitcast(F32R),
                    )

                with tc.tile_pool(name="es", bufs=D + 1) as es, \
                     tc.tile_pool(name="rp", bufs=2) as rp, \
                     tc.tile_pool(name="ob", bufs=3) as obp, \
                     tc.tile_pool(name="pss", bufs=2, space="PSUM") as pss, \
                     tc.tile_pool(name="pso", bufs=2, space="PSUM") as pso:

                    def proj_group(tt, oc):
                        po = pso.tile([128, 512], F32, name="po", tag="o")
                        for g in range(4):
                            nc.tensor.matmul(
                                out=po[:],
                                lhsT=YT[:, g * T + tt * 128: g * T + tt * 128 + 128],
                                rhs=Wp_sb[:, g * C + oc * 512: g * C + oc * 512 + 512],
                                start=(g == 0), stop=(g == 3),
                            )
                        ob = obp.tile([128, 512], F32, name="ob", tag="ob")
                        nc.vector.tensor_copy(ob[:], po[:])
                        nc.sync.dma_start(
                            out=out_d[tt * 128:(tt + 1) * 128,
                                      oc * 512:(oc + 1) * 512],
                            in_=ob[:],
                        )

                    prev_qc = None
                    for qc in (3, 2, 1, 0):
                        q0 = qc * 512
                        kmax = 4 * qc + 4
                        # proj groups of the previously finished q-chunk
                        pending = ([(tt, oc) for tt in range(4 * prev_qc, 4 * prev_qc + 4)
                                    for oc in range(2)] if prev_qc is not None else [])
                        for g in range(4):
                            gq = g * T
                            oAB = pso.tile([65, 1024], F32, name="oAB", tag="o")
                            e_l = [None] * kmax
                            off_l = [None] * kmax
                            for step in range(kmax + D):
                                if step < kmax:
                                    kt = step
                                    k0 = kt * 128
                                    toff = 128 * (kt - 4 * qc) if kt >= 4 * qc else 0
                                    w = 512 - toff
                                    off_l[kt] = toff
                                    diag = kt >= 4 * qc
                                    sAB = pss.tile([128, 1024], F32, name="sAB", tag="sAB")
                                    nc.tensor.matmul(
                                        out=sAB[:, toff:512],
                                        lhsT=KT[0:64, gq + k0: gq + k0 + 128],
                                        rhs=QT[0:64, gq + q0 + toff: gq + q0 + 512],
                                        start=True, stop=not diag,
                                    )
                                    nc.tensor.matmul(
                                        out=sAB[:, 512 + toff:1024],
                                        lhsT=KT[64:128, gq + k0: gq + k0 + 128],
                                        rhs=QT[64:128, gq + q0 + toff: gq + q0 + 512],
                                        start=True, stop=not diag, tile_position=(64, 0),
                                    )
                                    if diag:
                                        # additive causal mask on the 128-wide
                                        # diagonal band, via identity matmul
                                        nc.tensor.matmul(
                                            out=sAB[:, toff:toff + 128],
                                            lhsT=ident_sb[:],
                                            rhs=mask_sb[:],
                                            start=False, stop=True,
                                        )
                                        nc.tensor.matmul(
                                            out=sAB[:, 512 + toff:512 + toff + 128],
                                            lhsT=ident_sb[:],
                                            rhs=mask_sb[:],
                                            start=False, stop=True,
                                        )
                                    eAB = es.tile([128, 1024], BF16, name="eAB", tag="eAB")
                                    s3 = sAB.rearrange("p (h w) -> p h w", h=2, w=512)
                                    e3 = eAB.rearrange("p (h w) -> p h w", h=2, w=512)
                                    nc.scalar.activation(
                                        e3[:, :, toff:512], s3[:, :, toff:512], AF.Exp,
                                        bias=pad_sb[:, kt:kt + 1])
                                    e_l[kt] = eAB
                                pv = step - D
                                if 0 <= pv < kmax:
                                    toff = off_l[pv]
                                    vbase = pv * 520
                                    nc.tensor.matmul(
                                        out=oAB[:, toff:512],
                                        lhsT=Vt[:, vbase + 130 * g: vbase + 130 * g + 65],
                                        rhs=e_l[pv][:, toff:512],
                                        start=(pv == 0), stop=(pv == kmax - 1),
                                    )
                                    nc.tensor.matmul(
                                        out=oAB[:, 512 + toff:1024],
                                        lhsT=Vt[:, vbase + 130 * g + 65: vbase + 130 * g + 130],
                                        rhs=e_l[pv][:, 512 + toff:1024],
                                        start=(pv == 0), stop=(pv == kmax - 1),
                                    )
                            # epilogue: normalize by rowsum (row 64), write Y^T
                            rA = rp.tile([1, 512], F32, name="rA", tag="rA")
                            rB = rp.tile([1, 512], F32, name="rB", tag="rB")
                            nc.vector.reciprocal(rA[:], oAB[64:65, 0:512])
                            nc.vector.reciprocal(rB[:], oAB[64:65, 512:1024])
                            rbA = rp.tile([64, 512], F32, name="rbA", tag="rbA")
                            rbB = rp.tile([64, 512], F32, name="rbB", tag="rbB")
                            nc.gpsimd.partition_broadcast(rbA[:], rA[:])
                            nc.gpsimd.partition_broadcast(rbB[:], rB[:])
                            nc.vector.tensor_mul(
                                YT[0:64, gq + q0: gq + q0 + 512],
                                oAB[0:64, 0:512], rbA[:])
                            nc.vector.tensor_mul(
                                YT[64:128, gq + q0: gq + q0 + 512],
                                oAB[0:64, 512:1024], rbB[:])
                            # PE filler: two proj groups of the previous chunk
                            for _ in range(2):
                                if pending:
                                    proj_group(*pending.pop(0))
                        prev_qc = qc
                    # last processed chunk's projection
                    for tt in range(0, 4):
                        for oc in range(2):
                            proj_group(tt, oc)

    nc.compile()
    return nc


def _in_maps(x, Wk, bk, Wq, bq, Wv, bv, Wp, bp, padding_mask):
    maps = []
    mask_cols = np.arange(896)[None, :]
    mask_rows = np.arange(128)[:, None]
    maskneg = np.where(mask_rows <= mask_cols - 384, 0.0, NEG).astype(np.float32)
    for core in range(8):
        b, half = divmod(core, 2)
        hs = slice(half * IC, (half + 1) * IC)
        maps.append({
            "xT": np.ascontiguousarray(x[b].T),
            "WqT": np.ascontiguousarray(Wq[hs, :].T),
            "WkT": np.ascontiguousarray(Wk[hs, :].T),
            "WvT": np.ascontiguousarray(Wv[hs, :].T),
            "WpT": np.ascontiguousarray(Wp[:, hs].T),
            "bqs": np.ascontiguousarray((bq[hs] * SCALE).reshape(4, 128).T),
            "bks": np.ascontiguousarray(bk[hs].reshape(4, 128).T),
            "bvr": bv[hs].reshape(1, IC).copy(),
            "padb": np.ascontiguousarray(
                np.where(padding_mask[b] != 0, 0.0, NEG)
                .astype(np.float32).reshape(NKT, 128).T),
            "maskneg": maskneg,
            "ones128": np.ones((1, 128), np.float32),
            "ones8": np.ones((128, 8), ml_dtypes.bfloat16),
            "ident": np.eye(128).astype(ml_dtypes.bfloat16),
            "maskb": maskneg[:, 384:512].astype(ml_dtypes.bfloat16),
        })
    return maps


def _run(inputs, trace=False, **kw):
    if "nc" not in _CACHE:
        _CACHE["nc"] = _build()
    nc = _CACHE["nc"]
    ins = {k: np.asarray(v, dtype=np.float32) if k != "padding_mask"
           else np.asarray(v) for k, v in inputs.items()}
    maps = _in_maps(**ins)
    res = run_bass_kernel_spmd(nc, maps, core_ids=list(range(8)), trace=trace, **kw)
    bp = np.asarray(inputs["bp"], np.float32)
    y = np.empty((B, T, C), np.float32)
    for b in range(B):
        y[b] = res.results[2 * b]["out"] + res.results[2 * b + 1]["out"] + bp
    return y, res


def kernel(**inputs):
    y, _ = _run(inputs, trace=False)
    return y
